# revision 12
# baseline (speedup 1.0000x reference)
"""Trainium2 Bass kernel for nn_CrossAttention (B=2, Nq=Nk=2048, H=8, Dh=64,
Dx=512, Dctx=768).

Sharding: (batch, head-pair) across 8 cores — core c = (b, p) with b = c//4,
p = c%4 handles heads {2p, 2p+1} of batch b over ALL 2048 queries. K/V/Q
projections cover only the core's 128-wide D_inner slice (4x less projection
work than q-sharding); the output projection emits the PARTIAL product
Wo[128p:128p+128, :]^T @ attn_pair^T which the host sums across the 4 cores
of each batch during unshard (bias bo added on host).

Schedule is paced by ACT-engine exp (~72us/core floor at [128,1024] per
instruction): S = K^T Q runs as row-tiled PE pairs (tile_position
(0,0)/(64,0), both heads concurrent), each head's S lives in its own 2-bank
PSUM tile so next-round S matmuls chase exp bank-pair by bank-pair, softmax
denominators come from a ones-column in the V stationary ([128, 65]), and
normalization uses the custom-DVE approx reciprocal (valid only at partition
base 0 — denom row is first copied down from partition 64). DMA uses >=2KB
per-partition lines (whole xt tiles, ctxt half-tiles); K/V/Q projection tails
are spread thinly through qb0's attention rounds; each qb's normalize +
out-projection is deferred into the next qb's early rounds; the last qb runs
single-head rounds so one head's normalize overlaps the other's attention.
"""

import sys

sys.path.insert(0, "/opt/trn_rl_repo")

import numpy as np
import ml_dtypes

import concourse.bacc as bacc
import concourse.mybir as mybir
import concourse.tile as tile
from concourse.bass_utils import run_bass_kernel_spmd
from contextlib import ExitStack

F32 = mybir.dt.float32
BF16 = mybir.dt.bfloat16
NP_BF16 = np.dtype(ml_dtypes.bfloat16)

B = 2
NQ = 2048
NKV = 2048
DX = 512
DC = 768
DI = 512
NH = 8
DH = 64
DP = 128
N_CORES = 8

KC_X = DX // 128
KC_C = DC // 128
NKC = NKV // 128
NQB = NQ // 512
NRND = NKC // 2
SCALE = DH ** -0.5

_CACHE = {}


def _build_nc():
    nc = bacc.Bacc("TRN2", target_bir_lowering=False, debug=False, num_devices=N_CORES)

    xt = nc.declare_dram_parameter("xt", [DX, NQ], BF16, isOutput=False)
    ctxt = nc.declare_dram_parameter("ctxt", [DC, NKV], BF16, isOutput=False)
    wq = nc.declare_dram_parameter("wq", [DX, DP], BF16, isOutput=False)
    wk = nc.declare_dram_parameter("wk", [DC, DP], BF16, isOutput=False)
    wv = nc.declare_dram_parameter("wv", [DC, DP], BF16, isOutput=False)
    wo = nc.declare_dram_parameter("wo", [DP, DI], BF16, isOutput=False)
    otp = nc.declare_dram_parameter("otp", [DI, NQ], BF16, isOutput=True)

    with tile.TileContext(nc) as tc:
        with ExitStack() as ctx:
            # ---- SBUF pools ----
            const_p = ctx.enter_context(tc.tile_pool(name="const", bufs=1))
            w_p = ctx.enter_context(tc.tile_pool(name="weights", bufs=1))
            x_p = ctx.enter_context(tc.tile_pool(name="xt", bufs=1))
            ctx_p = ctx.enter_context(tc.tile_pool(name="ctxt", bufs=1))
            kt_p = ctx.enter_context(tc.tile_pool(name="kt", bufs=1))
            qt_p = ctx.enter_context(tc.tile_pool(name="qt", bufs=1))
            va_p = ctx.enter_context(tc.tile_pool(name="va", bufs=1))
            p_p = ctx.enter_context(tc.tile_pool(name="p", bufs=6))
            at_p = ctx.enter_context(tc.tile_pool(name="at", bufs=2))
            small_p = ctx.enter_context(tc.tile_pool(name="small", bufs=2))
            out_p = ctx.enter_context(tc.tile_pool(name="outsb", bufs=2))
            # ---- PSUM pools: (2+2) + 2 + 2 = 8 banks ----
            sa_ps = ctx.enter_context(tc.tile_pool(name="sa", bufs=1, space="PSUM"))
            sb_ps = ctx.enter_context(tc.tile_pool(name="sb", bufs=1, space="PSUM"))
            pv_ps = ctx.enter_context(tc.tile_pool(name="pv", bufs=2, space="PSUM"))
            proj_ps = ctx.enter_context(tc.tile_pool(name="proj", bufs=2, space="PSUM"))

            # ---- constants; dummy exp preloads the ACT exp table set ----
            ones_f = const_p.tile([1, 64], F32)
            nc.any.memset(ones_f[:], 1.0)
            ones_r = const_p.tile([1, 64], BF16)
            nc.vector.tensor_copy(ones_r[:], ones_f[:])
            ones32 = const_p.tile([128, 32], F32)
            nc.any.memset(ones32[:], 1.0)
            warm = const_p.tile([1, 16], F32)
            nc.any.memset(warm[:], 0.0)
            warm_o = const_p.tile([1, 16], BF16)
            nc.scalar.activation(warm_o[:], warm[:],
                                 mybir.ActivationFunctionType.Exp, scale=1.0)

            # ---- DMA: >=2KB per-partition lines; round-0 inputs first ----
            wq_t = []
            for c in range(KC_X):
                t = w_p.tile([128, DP], BF16, tag=f"wq{c}")
                nc.sync.dma_start(t[:], wq[c * 128:(c + 1) * 128, :])
                wq_t.append(t)
            wk_t = []
            for c in range(KC_C):
                t = w_p.tile([128, DP], BF16, tag=f"wk{c}")
                nc.sync.dma_start(t[:], wk[c * 128:(c + 1) * 128, :])
                wk_t.append(t)
            wv_t = []
            for c in range(KC_C):
                t = w_p.tile([128, DP], BF16, tag=f"wv{c}")
                nc.sync.dma_start(t[:], wv[c * 128:(c + 1) * 128, :])
                wv_t.append(t)
            wo_t = w_p.tile([128, DI], BF16, tag="wo")
            nc.sync.dma_start(wo_t[:], wo[:, :])

            xt_t = []
            for c in range(KC_X):
                t = x_p.tile([128, NQ], BF16, tag=f"xt{c}", name=f"xt{c}")
                nc.sync.dma_start(t[:], xt[c * 128:(c + 1) * 128, :])
                xt_t.append(t)
            ctx_t = [ctx_p.tile([128, NKV], BF16, tag=f"ctx{c}", name=f"ctx{c}")
                     for c in range(KC_C)]
            for half in range(2):
                for c in range(KC_C):
                    nc.sync.dma_start(
                        ctx_t[c][:, half * 1024:(half + 1) * 1024],
                        ctxt[c * 128:(c + 1) * 128, half * 1024:(half + 1) * 1024])

            # ---- projection emitters ----
            qt_t = qt_p.tile([128, NQ], BF16)
            kt_t = kt_p.tile([128, NKV], BF16)
            va_t = va_p.tile([128, NKC * 130], BF16)
            dst_ones = va_t[:].rearrange("p (g c) -> p g c", c=65)[:, :, 64:65]
            nc.vector.tensor_copy(dst_ones, ones32[:, :, None])

            def emit_q_qb(qb):
                ps = proj_ps.tile([128, 512], F32, tag="proj", name=f"pq{qb}")
                for c in range(KC_X):
                    nc.tensor.matmul(ps[:], wq_t[c][:],
                                     xt_t[c][:, qb * 512:(qb + 1) * 512],
                                     start=(c == 0), stop=(c == KC_X - 1))
                nc.vector.tensor_copy(qt_t[:, qb * 512:(qb + 1) * 512], ps[:])

            def emit_k_kvb(kvb):
                ps = proj_ps.tile([128, 512], F32, tag="proj", name=f"pk{kvb}")
                for c in range(KC_C):
                    nc.tensor.matmul(ps[:], wk_t[c][:],
                                     ctx_t[c][:, kvb * 512:(kvb + 1) * 512],
                                     start=(c == 0), stop=(c == KC_C - 1))
                nc.vector.tensor_copy(kt_t[:, kvb * 512:(kvb + 1) * 512], ps[:])

            def emit_v_pair(vp):
                # two kv-chunks (2*vp, 2*vp+1) -> one [128, 256] psum region
                ps = proj_ps.tile([128, 512], F32, tag="proj", name=f"pvg{vp}")
                for i in range(2):
                    kvc = vp * 2 + i
                    for c in range(KC_C):
                        nc.tensor.matmul(
                            ps[:, i * 128:(i + 1) * 128],
                            ctx_t[c][:, kvc * 128:(kvc + 1) * 128], wv_t[c][:],
                            start=(c == 0), stop=(c == KC_C - 1))
                src = ps[:, 0:256].rearrange("p (i h d) -> p i h d", i=2, h=2)
                dst = va_t[:, vp * 260:(vp + 1) * 260]
                dst = dst.rearrange("p (i h d) -> p i h d", i=2, h=2, d=65)[:, :, :, 0:64]
                nc.vector.tensor_copy(dst, src)

            emit_q_qb(0)
            emit_k_kvb(0)
            emit_k_kvb(1)
            for vp in range(2):
                emit_v_pair(vp)

            # fillers after round r of qb0 (kt kvb k ready before S round 2k;
            # va chunk pair g ready before PV round g, which is emitted in
            # round g+1)
            qb0_fill = {
                0: [lambda: emit_v_pair(2), lambda: emit_v_pair(3)],
                1: [lambda: emit_k_kvb(2), lambda: emit_v_pair(4)],
                2: [lambda: emit_v_pair(5)],
                3: [lambda: emit_k_kvb(3), lambda: emit_v_pair(6)],
                4: [lambda: emit_v_pair(7), lambda: emit_q_qb(1)],
                5: [lambda: emit_q_qb(2)],
                6: [lambda: emit_q_qb(3)],
            }

            # ---- attention; norm + out-proj of qb deferred into qb+1 ----
            sps = [sa_ps, sb_ps]

            def make_tail(qb, pv_t, heads=(0, 1)):
                def tail_norm(at_t=None):
                    if at_t is None:
                        at_t = at_p.tile([128, 512], BF16, tag="at", name=f"at{qb}")
                    for h in heads:
                        den = small_p.tile([1, 512], F32, tag="den",
                                           name=f"den{qb}_{h}")
                        nc.vector.tensor_copy(den[:], pv_t[h][64:65, :])
                        rec = small_p.tile([1, 512], F32, tag="rec",
                                           name=f"rec{qb}_{h}")
                        nc.vector.reciprocal_approx_fast(rec[:], den[:])
                        rec_b = small_p.tile([1, 512], BF16, tag="recb",
                                             name=f"recb{qb}_{h}")
                        nc.vector.tensor_copy(rec_b[:], rec[:])
                        ps_b = proj_ps.tile([64, 512], F32, tag="proj",
                                            name=f"psb{qb}_{h}")
                        nc.tensor.matmul(ps_b[:], ones_r[:], rec_b[:],
                                         start=True, stop=True)
                        b_sb = small_p.tile([64, 512], F32, tag="bsb",
                                            name=f"bsb{qb}_{h}")
                        nc.vector.tensor_copy(b_sb[:], ps_b[:])
                        nc.vector.tensor_tensor(at_t[h * 64:(h + 1) * 64, :],
                                                pv_t[h][0:64, :], b_sb[:],
                                                op=mybir.AluOpType.mult)
                    return at_t

                def tail_oproj(at_t):
                    for m in range(4):
                        ps_o = proj_ps.tile([128, 512], F32, tag="proj",
                                            name=f"po{qb}_{m}")
                        nc.tensor.matmul(ps_o[:], wo_t[:, m * 128:(m + 1) * 128],
                                         at_t[:], start=True, stop=True)
                        o_sb = out_p.tile([128, 512], BF16, tag="osb",
                                          name=f"o{qb}_{m}")
                        nc.vector.tensor_copy(o_sb[:], ps_o[:])
                        nc.sync.dma_start(
                            otp[m * 128:(m + 1) * 128, qb * 512:(qb + 1) * 512],
                            o_sb[:])

                return tail_norm, tail_oproj

            pending_tail = [None]

            def drain_tail(step):
                # step 0: run deferred norm; step 1: run deferred out-proj
                if pending_tail[0] is None:
                    return
                if step == 0:
                    norm, oproj = pending_tail[0]
                    pending_tail[0] = (norm(), oproj)
                else:
                    at_prev, oproj = pending_tail[0]
                    oproj(at_prev)
                    pending_tail[0] = None

            def s_head(qb, g, h, sp_name):
                sp = sps[h].tile([128, 1024], F32, tag=f"s{h}", name=sp_name)
                for j in range(2):
                    kvc = g * 2 + j
                    nc.tensor.matmul(
                        sp[:, j * 512:(j + 1) * 512],
                        kt_t[h * 64:(h + 1) * 64, kvc * 128:(kvc + 1) * 128],
                        qt_t[h * 64:(h + 1) * 64, qb * 512:(qb + 1) * 512],
                        start=True, stop=True)
                p_t = p_p.tile([128, 1024], BF16, tag="p", name=f"p_{sp_name}")
                nc.scalar.activation(p_t[:], sp[:],
                                     mybir.ActivationFunctionType.Exp, scale=SCALE)
                return p_t

            def pv_head(pv_t, va_col, p_t):
                for j in range(2):
                    kvc = va_col[j]
                    nc.tensor.matmul(
                        pv_t[:],
                        va_t[:, kvc * 130:kvc * 130 + 65],
                        p_t[:, j * 512:(j + 1) * 512],
                        start=(kvc == 0), stop=(kvc == NKC - 1))

            def pv_head_h(pv_t, g, h, p_t):
                for j in range(2):
                    kvc = g * 2 + j
                    nc.tensor.matmul(
                        pv_t[:],
                        va_t[:, kvc * 130 + h * 65:kvc * 130 + (h + 1) * 65],
                        p_t[:, j * 512:(j + 1) * 512],
                        start=(kvc == 0), stop=(kvc == NKC - 1))

            def emit_attn_qb(qb):
                pv_t = [pv_ps.tile([65, 512], F32, tag="pv", name=f"pv{qb}_{h}")
                        for h in range(2)]
                p_prev = None
                for g in range(NRND):
                    p_cur = [s_head(qb, g, 0, f"s{qb}_{g}_0"),
                             s_head(qb, g, 1, f"s{qb}_{g}_1")]
                    if qb == 0 and g in qb0_fill:
                        for f in qb0_fill[g]:
                            f()
                    if g == 0:
                        drain_tail(0)
                    if g == 1:
                        drain_tail(1)
                    if p_prev is not None:
                        for h in range(2):
                            pv_head_h(pv_t[h], g - 1, h, p_prev[h])
                    p_prev = p_cur
                for h in range(2):
                    pv_head_h(pv_t[h], NRND - 1, h, p_prev[h])
                pending_tail[0] = make_tail(qb, pv_t)

            def emit_attn_qb_split(qb):
                # last q-block: single-head rounds; head 0's normalize
                # overlaps head 1's attention, shrinking the final tail
                pv_t = [pv_ps.tile([65, 512], F32, tag="pv", name=f"pv{qb}_{h}")
                        for h in range(2)]
                at_t = at_p.tile([128, 512], BF16, tag="at", name=f"at{qb}")
                norm0, _ = make_tail(qb, pv_t, heads=(0,))
                norm1, oproj = make_tail(qb, pv_t, heads=(1,))
                for h in range(2):
                    p_prev = None
                    for g in range(NRND):
                        p_cur = s_head(qb, g, h, f"s{qb}_{g}_{h}")
                        if h == 0 and g == 0:
                            drain_tail(0)
                        if h == 0 and g == 1:
                            drain_tail(1)
                        if h == 1 and g == 1:
                            norm0(at_t)  # head 0 normalize under head 1 rounds
                        if p_prev is not None:
                            pv_head_h(pv_t[h], g - 1, h, p_prev)
                        p_prev = p_cur
                    pv_head_h(pv_t[h], NRND - 1, h, p_prev)
                norm1(at_t)
                oproj(at_t)

            for qb in range(NQB - 1):
                emit_attn_qb(qb)
            emit_attn_qb_split(NQB - 1)

    nc.finalize()
    return nc


def _bf16(a):
    return np.ascontiguousarray(a).astype(NP_BF16)


def run_spmd(inputs, trace=False):
    if "nc" not in _CACHE:
        _CACHE["nc"] = _build_nc()
    nc = _CACHE["nc"]

    x = np.asarray(inputs["x"], dtype=np.float32)
    context = np.asarray(inputs["context"], dtype=np.float32)
    Wq = np.asarray(inputs["Wq"], dtype=np.float32)
    Wk = np.asarray(inputs["Wk"], dtype=np.float32)
    Wv = np.asarray(inputs["Wv"], dtype=np.float32)
    Wo = np.asarray(inputs["Wo"], dtype=np.float32)
    bo = np.asarray(inputs["bo"], dtype=np.float32)

    xt_b = [_bf16(x[b].T) for b in range(B)]
    ctxt_b = [_bf16(context[b].T) for b in range(B)]
    in_maps = []
    for c in range(N_CORES):
        b, p = c // 4, c % 4
        s = slice(p * DP, (p + 1) * DP)
        in_maps.append({
            "xt": xt_b[b], "ctxt": ctxt_b[b],
            "wq": _bf16(Wq[:, s]), "wk": _bf16(Wk[:, s]), "wv": _bf16(Wv[:, s]),
            "wo": _bf16(Wo[s, :]),
        })

    res = run_bass_kernel_spmd(nc, in_maps, core_ids=list(range(N_CORES)),
                               trace=trace)
    out = np.empty((B, NQ, DI), dtype=np.float32)
    for b in range(B):
        acc = np.zeros((DI, NQ), dtype=np.float32)
        for p in range(4):
            acc += res.results[4 * b + p]["otp"].astype(np.float32)
        out[b] = acc.T + bo[None, :]
    return out, res


def kernel(**inputs):
    out, _ = run_spmd(inputs, trace=False)
    return out


# revision 14
# speedup vs baseline: 1.0128x; 1.0128x over previous
"""Trainium2 Bass kernel for nn_CrossAttention (B=2, Nq=Nk=2048, H=8, Dh=64,
Dx=512, Dctx=768).

Sharding: (batch, head-pair) across 8 cores — core c = (b, p) with b = c//4,
p = c%4 handles heads {2p, 2p+1} of batch b over ALL 2048 queries. K/V/Q
projections cover only the core's 128-wide D_inner slice (4x less projection
work than q-sharding); the output projection emits the PARTIAL product
Wo[128p:128p+128, :]^T @ attn_pair^T which the host sums across the 4 cores
of each batch during unshard (bias bo added on host).

Schedule is paced by ACT-engine exp (~72us/core floor at [128,1024] per
instruction): S = K^T Q runs as row-tiled PE pairs (tile_position
(0,0)/(64,0), both heads concurrent), each head's S lives in its own 2-bank
PSUM tile so next-round S matmuls chase exp bank-pair by bank-pair, softmax
denominators come from a ones-column in the V stationary ([128, 65]), and
normalization uses the custom-DVE approx reciprocal (valid only at partition
base 0 — denom row is first copied down from partition 64). DMA uses >=2KB
per-partition lines (weights packed into one [128, 2560] transfer, whole xt
tiles, ctxt half-tiles); K/V/Q projection tails are spread thinly through
qb0's attention rounds; each qb's normalize + out-projection is deferred in
three stages into the next qb's early rounds so neither ACT nor the PE queue
stalls at block boundaries.
"""

import sys

sys.path.insert(0, "/opt/trn_rl_repo")

import numpy as np
import ml_dtypes

import concourse.bacc as bacc
import concourse.mybir as mybir
import concourse.tile as tile
from concourse.bass_utils import run_bass_kernel_spmd
from contextlib import ExitStack

F32 = mybir.dt.float32
BF16 = mybir.dt.bfloat16
NP_BF16 = np.dtype(ml_dtypes.bfloat16)

B = 2
NQ = 2048
NKV = 2048
DX = 512
DC = 768
DI = 512
NH = 8
DH = 64
DP = 128
N_CORES = 8

KC_X = DX // 128
KC_C = DC // 128
NKC = NKV // 128
NQB = NQ // 512
NRND = NKC // 2
SCALE = DH ** -0.5

_CACHE = {}


def _build_nc():
    nc = bacc.Bacc("TRN2", target_bir_lowering=False, debug=False, num_devices=N_CORES)

    xt = nc.declare_dram_parameter("xt", [DX, NQ], BF16, isOutput=False)
    ctxt = nc.declare_dram_parameter("ctxt", [DC, NKV], BF16, isOutput=False)
    wpack = nc.declare_dram_parameter("wpack", [128, 2560], BF16, isOutput=False)
    otp = nc.declare_dram_parameter("otp", [DI, NQ], BF16, isOutput=True)

    with tile.TileContext(nc) as tc:
        with ExitStack() as ctx:
            # ---- SBUF pools ----
            const_p = ctx.enter_context(tc.tile_pool(name="const", bufs=1))
            w_p = ctx.enter_context(tc.tile_pool(name="weights", bufs=1))
            x_p = ctx.enter_context(tc.tile_pool(name="xt", bufs=1))
            ctx_p = ctx.enter_context(tc.tile_pool(name="ctxt", bufs=1))
            kt_p = ctx.enter_context(tc.tile_pool(name="kt", bufs=1))
            qt_p = ctx.enter_context(tc.tile_pool(name="qt", bufs=1))
            va_p = ctx.enter_context(tc.tile_pool(name="va", bufs=1))
            p_p = ctx.enter_context(tc.tile_pool(name="p", bufs=6))
            at_p = ctx.enter_context(tc.tile_pool(name="at", bufs=2))
            small_p = ctx.enter_context(tc.tile_pool(name="small", bufs=2))
            out_p = ctx.enter_context(tc.tile_pool(name="outsb", bufs=2))
            # ---- PSUM pools: (2+2) + 2 + 2 = 8 banks ----
            sa_ps = ctx.enter_context(tc.tile_pool(name="sa", bufs=1, space="PSUM"))
            sb_ps = ctx.enter_context(tc.tile_pool(name="sb", bufs=1, space="PSUM"))
            pv_ps = ctx.enter_context(tc.tile_pool(name="pv", bufs=2, space="PSUM"))
            proj_ps = ctx.enter_context(tc.tile_pool(name="proj", bufs=2, space="PSUM"))

            # ---- constants; dummy exp preloads the ACT exp table set ----
            ones_f = const_p.tile([1, 64], F32)
            nc.any.memset(ones_f[:], 1.0)
            ones_r = const_p.tile([1, 64], BF16)
            nc.vector.tensor_copy(ones_r[:], ones_f[:])
            ones32 = const_p.tile([128, 32], F32)
            nc.any.memset(ones32[:], 1.0)
            warm = const_p.tile([1, 16], F32)
            nc.any.memset(warm[:], 0.0)
            warm_o = const_p.tile([1, 16], BF16)
            nc.scalar.activation(warm_o[:], warm[:],
                                 mybir.ActivationFunctionType.Exp, scale=1.0)

            # ---- DMA: >=2KB per-partition lines; round-0 inputs first ----
            # all weights arrive as ONE [128, 2560] DMA (5KB partition lines;
            # per-tile weight DMAs were 256B lines and crawled)
            w_t = w_p.tile([128, 2560], BF16, tag="wpack")
            nc.sync.dma_start(w_t[:], wpack[:, :])
            wq_t = [w_t[:, c * 128:(c + 1) * 128] for c in range(KC_X)]
            wk_t = [w_t[:, 512 + c * 128:512 + (c + 1) * 128] for c in range(KC_C)]
            wv_t = [w_t[:, 1280 + c * 128:1280 + (c + 1) * 128] for c in range(KC_C)]
            wo_t = w_t[:, 2048:2560]

            ctx_t = [ctx_p.tile([128, NKV], BF16, tag=f"ctx{c}", name=f"ctx{c}")
                     for c in range(KC_C)]
            xt_t = [x_p.tile([128, NQ], BF16, tag=f"xt{c}", name=f"xt{c}")
                    for c in range(KC_X)]
            # order: ctxt first half (K kvb0/kvb1 + V inputs), then xt, then rest
            for c in range(KC_C):
                nc.sync.dma_start(ctx_t[c][:, 0:1024], ctxt[c * 128:(c + 1) * 128, 0:1024])
            for c in range(KC_X):
                nc.sync.dma_start(xt_t[c][:], xt[c * 128:(c + 1) * 128, :])
            for c in range(KC_C):
                nc.sync.dma_start(ctx_t[c][:, 1024:2048],
                                  ctxt[c * 128:(c + 1) * 128, 1024:2048])

            # ---- projection emitters ----
            qt_t = qt_p.tile([128, NQ], BF16)
            kt_t = kt_p.tile([128, NKV], BF16)
            va_t = va_p.tile([128, NKC * 130], BF16)
            dst_ones = va_t[:].rearrange("p (g c) -> p g c", c=65)[:, :, 64:65]
            nc.vector.tensor_copy(dst_ones, ones32[:, :, None])

            def emit_q_qb(qb):
                ps = proj_ps.tile([128, 512], F32, tag="proj", name=f"pq{qb}")
                for c in range(KC_X):
                    nc.tensor.matmul(ps[:], wq_t[c],
                                     xt_t[c][:, qb * 512:(qb + 1) * 512],
                                     start=(c == 0), stop=(c == KC_X - 1))
                nc.vector.tensor_copy(qt_t[:, qb * 512:(qb + 1) * 512], ps[:])

            def emit_k_kvb(kvb):
                ps = proj_ps.tile([128, 512], F32, tag="proj", name=f"pk{kvb}")
                for c in range(KC_C):
                    nc.tensor.matmul(ps[:], wk_t[c],
                                     ctx_t[c][:, kvb * 512:(kvb + 1) * 512],
                                     start=(c == 0), stop=(c == KC_C - 1))
                nc.vector.tensor_copy(kt_t[:, kvb * 512:(kvb + 1) * 512], ps[:])

            def emit_v_pair(vp):
                # two kv-chunks (2*vp, 2*vp+1) -> one [128, 256] psum region
                ps = proj_ps.tile([128, 512], F32, tag="proj", name=f"pvg{vp}")
                for i in range(2):
                    kvc = vp * 2 + i
                    for c in range(KC_C):
                        nc.tensor.matmul(
                            ps[:, i * 128:(i + 1) * 128],
                            ctx_t[c][:, kvc * 128:(kvc + 1) * 128], wv_t[c],
                            start=(c == 0), stop=(c == KC_C - 1))
                src = ps[:, 0:256].rearrange("p (i h d) -> p i h d", i=2, h=2)
                dst = va_t[:, vp * 260:(vp + 1) * 260]
                dst = dst.rearrange("p (i h d) -> p i h d", i=2, h=2, d=65)[:, :, :, 0:64]
                nc.vector.tensor_copy(dst, src)

            emit_q_qb(0)
            emit_k_kvb(0)
            emit_k_kvb(1)
            for vp in range(2):
                emit_v_pair(vp)

            # fillers after round r of qb0 (kt kvb k ready before S round 2k;
            # va chunk pair g ready before PV round g, which is emitted in
            # round g+1)
            qb0_fill = {
                0: [lambda: emit_v_pair(2), lambda: emit_v_pair(3)],
                1: [lambda: emit_k_kvb(2), lambda: emit_v_pair(4)],
                2: [lambda: emit_v_pair(5)],
                3: [lambda: emit_k_kvb(3), lambda: emit_v_pair(6)],
                4: [lambda: emit_v_pair(7), lambda: emit_q_qb(1)],
                5: [lambda: emit_q_qb(2)],
                6: [lambda: emit_q_qb(3)],
            }

            # ---- attention; norm + out-proj of qb deferred into qb+1 ----
            sps = [sa_ps, sb_ps]

            def make_tail(qb, pv_t):
                recs = [None, None]

                def norm_pre():
                    # DVE-only: denom row down to partition 0, approx recip
                    for h in range(2):
                        den = small_p.tile([1, 512], F32, tag="den",
                                           name=f"den{qb}_{h}")
                        nc.vector.tensor_copy(den[:], pv_t[h][64:65, :])
                        rec = small_p.tile([1, 512], F32, tag="rec",
                                           name=f"rec{qb}_{h}")
                        nc.vector.reciprocal_approx_fast(rec[:], den[:])
                        rec_b = small_p.tile([1, 512], BF16, tag="recb",
                                             name=f"recb{qb}_{h}")
                        nc.vector.tensor_copy(rec_b[:], rec[:])
                        recs[h] = rec_b

                def norm_fin():
                    at_t = at_p.tile([128, 512], BF16, tag="at", name=f"at{qb}")
                    for h in range(2):
                        ps_b = proj_ps.tile([64, 512], F32, tag="proj",
                                            name=f"psb{qb}_{h}")
                        nc.tensor.matmul(ps_b[:], ones_r[:], recs[h][:],
                                         start=True, stop=True)
                        b_sb = small_p.tile([64, 512], F32, tag="bsb",
                                            name=f"bsb{qb}_{h}")
                        nc.vector.tensor_copy(b_sb[:], ps_b[:])
                        nc.vector.tensor_tensor(at_t[h * 64:(h + 1) * 64, :],
                                                pv_t[h][0:64, :], b_sb[:],
                                                op=mybir.AluOpType.mult)
                    return at_t

                def tail_oproj(at_t, final=False):
                    for m in range(4):
                        ps_o = proj_ps.tile([128, 512], F32, tag="proj",
                                            name=f"po{qb}_{m}")
                        nc.tensor.matmul(ps_o[:], wo_t[:, m * 128:(m + 1) * 128],
                                         at_t[:], start=True, stop=True)
                        o_sb = out_p.tile([128, 512], BF16, tag="osb",
                                          name=f"o{qb}_{m}")
                        if final and m % 2 == 1:
                            # ACT is idle after the last exp; share the copies
                            nc.scalar.copy(o_sb[:], ps_o[:])
                        else:
                            nc.vector.tensor_copy(o_sb[:], ps_o[:])
                        nc.sync.dma_start(
                            otp[m * 128:(m + 1) * 128, qb * 512:(qb + 1) * 512],
                            o_sb[:])

                return [norm_pre, norm_fin, tail_oproj]

            pending_tail = [None]
            tail_at = [None]

            def drain_tail(step):
                # 0: DVE recip prefix; 1: broadcast+mult (before new PV r0);
                # 2: out-projection
                if pending_tail[0] is None:
                    return
                pre, fin, oproj = pending_tail[0]
                if step == 0:
                    pre()
                elif step == 1:
                    tail_at[0] = fin()
                else:
                    oproj(tail_at[0])
                    pending_tail[0] = None

            def s_head(qb, g, h, sp_name):
                sp = sps[h].tile([128, 1024], F32, tag=f"s{h}", name=sp_name)
                for j in range(2):
                    kvc = g * 2 + j
                    nc.tensor.matmul(
                        sp[:, j * 512:(j + 1) * 512],
                        kt_t[h * 64:(h + 1) * 64, kvc * 128:(kvc + 1) * 128],
                        qt_t[h * 64:(h + 1) * 64, qb * 512:(qb + 1) * 512],
                        start=True, stop=True)
                p_t = p_p.tile([128, 1024], BF16, tag="p", name=f"p_{sp_name}")
                nc.scalar.activation(p_t[:], sp[:],
                                     mybir.ActivationFunctionType.Exp, scale=SCALE)
                return p_t

            def pv_head(pv_t, va_col, p_t):
                for j in range(2):
                    kvc = va_col[j]
                    nc.tensor.matmul(
                        pv_t[:],
                        va_t[:, kvc * 130:kvc * 130 + 65],
                        p_t[:, j * 512:(j + 1) * 512],
                        start=(kvc == 0), stop=(kvc == NKC - 1))

            def pv_head_h(pv_t, g, h, p_t):
                for j in range(2):
                    kvc = g * 2 + j
                    nc.tensor.matmul(
                        pv_t[:],
                        va_t[:, kvc * 130 + h * 65:kvc * 130 + (h + 1) * 65],
                        p_t[:, j * 512:(j + 1) * 512],
                        start=(kvc == 0), stop=(kvc == NKC - 1))

            def emit_attn_qb(qb):
                pv_t = [pv_ps.tile([65, 512], F32, tag="pv", name=f"pv{qb}_{h}")
                        for h in range(2)]
                p_prev = None
                for g in range(NRND):
                    p_cur = [s_head(qb, g, 0, f"s{qb}_{g}_0"),
                             s_head(qb, g, 1, f"s{qb}_{g}_1")]
                    if qb == 0 and g in qb0_fill:
                        for f in qb0_fill[g]:
                            f()
                    if g <= 2:
                        drain_tail(g)
                    if p_prev is not None:
                        for h in range(2):
                            pv_head_h(pv_t[h], g - 1, h, p_prev[h])
                    p_prev = p_cur
                for h in range(2):
                    pv_head_h(pv_t[h], NRND - 1, h, p_prev[h])
                pending_tail[0] = make_tail(qb, pv_t)

            for qb in range(NQB):
                emit_attn_qb(qb)
            pre, fin, oproj = pending_tail[0]
            pre()
            oproj(fin(), final=True)

    nc.finalize()
    return nc


def _bf16(a):
    return np.ascontiguousarray(a).astype(NP_BF16)


def run_spmd(inputs, trace=False):
    if "nc" not in _CACHE:
        _CACHE["nc"] = _build_nc()
    nc = _CACHE["nc"]

    x = np.asarray(inputs["x"], dtype=np.float32)
    context = np.asarray(inputs["context"], dtype=np.float32)
    Wq = np.asarray(inputs["Wq"], dtype=np.float32)
    Wk = np.asarray(inputs["Wk"], dtype=np.float32)
    Wv = np.asarray(inputs["Wv"], dtype=np.float32)
    Wo = np.asarray(inputs["Wo"], dtype=np.float32)
    bo = np.asarray(inputs["bo"], dtype=np.float32)

    xt_b = [_bf16(x[b].T) for b in range(B)]
    ctxt_b = [_bf16(context[b].T) for b in range(B)]
    in_maps = []
    for c in range(N_CORES):
        b, p = c // 4, c % 4
        s = slice(p * DP, (p + 1) * DP)
        wpk = np.empty((128, 2560), dtype=NP_BF16)
        for c in range(KC_X):
            wpk[:, c * 128:(c + 1) * 128] = _bf16(Wq[c * 128:(c + 1) * 128, s])
        for c in range(KC_C):
            wpk[:, 512 + c * 128:512 + (c + 1) * 128] = \
                _bf16(Wk[c * 128:(c + 1) * 128, s])
            wpk[:, 1280 + c * 128:1280 + (c + 1) * 128] = \
                _bf16(Wv[c * 128:(c + 1) * 128, s])
        wpk[:, 2048:2560] = _bf16(Wo[s, :])
        in_maps.append({"xt": xt_b[b], "ctxt": ctxt_b[b], "wpack": wpk})

    res = run_bass_kernel_spmd(nc, in_maps, core_ids=list(range(N_CORES)),
                               trace=trace)
    out = np.empty((B, NQ, DI), dtype=np.float32)
    for b in range(B):
        acc = np.zeros((DI, NQ), dtype=np.float32)
        for p in range(4):
            acc += res.results[4 * b + p]["otp"].astype(np.float32)
        out[b] = acc.T + bo[None, :]
    return out, res


def kernel(**inputs):
    out, _ = run_spmd(inputs, trace=False)
    return out


# revision 17
# speedup vs baseline: 1.1599x; 1.1452x over previous
"""Trainium2 Bass kernel for nn_CrossAttention (B=2, Nq=Nk=2048, H=8, Dh=64,
Dx=512, Dctx=768).

Sharding: (batch, head-pair) across 8 cores — core c = (b, p) with b = c//4,
p = c%4 handles heads {2p, 2p+1} of batch b over ALL 2048 queries. K/V/Q
projections cover only the core's 128-wide D_inner slice (4x less projection
work than q-sharding); the output projection emits the PARTIAL product
Wo[128p:128p+128, :]^T @ attn_pair^T which the host sums across the 4 cores
of each batch during unshard (bias bo added on host).

Schedule is paced by ACT-engine exp (~72us/core floor at [128,1024] per
instruction): S = K^T Q runs as row-tiled PE pairs (tile_position
(0,0)/(64,0), both heads concurrent), each head's S lives in its own 2-bank
PSUM tile so next-round S matmuls chase exp bank-pair by bank-pair, softmax
denominators come from a ones-column in the V stationary ([128, 65]), and
normalization uses the custom-DVE approx reciprocal (valid only at partition
base 0 — denom row is first copied down from partition 64). DMA uses >=2KB
per-partition lines (weights packed into one [128, 2560] transfer, whole xt
tiles, ctxt half-tiles); K/V/Q projection tails are spread thinly through
qb0's attention rounds; each qb's normalize + out-projection is deferred in
three stages into the next qb's early rounds so neither ACT nor the PE queue
stalls at block boundaries.
"""

import sys

sys.path.insert(0, "/opt/trn_rl_repo")

import numpy as np
import ml_dtypes

import concourse.bacc as bacc
import concourse.mybir as mybir
import concourse.tile as tile
from concourse.bass_utils import run_bass_kernel_spmd
from contextlib import ExitStack

F32 = mybir.dt.float32
BF16 = mybir.dt.bfloat16
NP_BF16 = np.dtype(ml_dtypes.bfloat16)

B = 2
NQ = 2048
NKV = 2048
DX = 512
DC = 768
DI = 512
NH = 8
DH = 64
DP = 128
N_CORES = 8

KC_X = DX // 128
KC_C = DC // 128
NKC = NKV // 128
NQB = NQ // 512
NRND = NKC // 2
SCALE = DH ** -0.5

_CACHE = {}


def _build_nc():
    nc = bacc.Bacc("TRN2", target_bir_lowering=False, debug=False, num_devices=N_CORES)

    xt = nc.declare_dram_parameter("xt", [DX, NQ], BF16, isOutput=False)
    ctxt = nc.declare_dram_parameter("ctxt", [DC, NKV], BF16, isOutput=False)
    wpack = nc.declare_dram_parameter("wpack", [128, 2560], BF16, isOutput=False)
    otp = nc.declare_dram_parameter("otp", [DI, NQ], BF16, isOutput=True)

    with tile.TileContext(nc) as tc:
        with ExitStack() as ctx:
            # ---- SBUF pools ----
            const_p = ctx.enter_context(tc.tile_pool(name="const", bufs=1))
            w_p = ctx.enter_context(tc.tile_pool(name="weights", bufs=1))
            x_p = ctx.enter_context(tc.tile_pool(name="xt", bufs=1))
            ctx_p = ctx.enter_context(tc.tile_pool(name="ctxt", bufs=1))
            kt_p = ctx.enter_context(tc.tile_pool(name="kt", bufs=1))
            qt_p = ctx.enter_context(tc.tile_pool(name="qt", bufs=1))
            va_p = ctx.enter_context(tc.tile_pool(name="va", bufs=1))
            p_p = ctx.enter_context(tc.tile_pool(name="p", bufs=6))
            at_p = ctx.enter_context(tc.tile_pool(name="at", bufs=2))
            small_p = ctx.enter_context(tc.tile_pool(name="small", bufs=2))
            out_p = ctx.enter_context(tc.tile_pool(name="outsb", bufs=2))
            # ---- PSUM pools: (2+2) + 2 + 2 = 8 banks ----
            sa_ps = ctx.enter_context(tc.tile_pool(name="sa", bufs=1, space="PSUM"))
            sb_ps = ctx.enter_context(tc.tile_pool(name="sb", bufs=1, space="PSUM"))
            pv_ps = ctx.enter_context(tc.tile_pool(name="pv", bufs=2, space="PSUM"))
            proj_ps = ctx.enter_context(tc.tile_pool(name="proj", bufs=2, space="PSUM"))

            # ---- constants; dummy exp preloads the ACT exp table set ----
            ones_f = const_p.tile([1, 64], F32)
            nc.any.memset(ones_f[:], 1.0)
            ones_r = const_p.tile([1, 64], BF16)
            nc.vector.tensor_copy(ones_r[:], ones_f[:])
            ones32 = const_p.tile([128, 32], F32)
            nc.any.memset(ones32[:], 1.0)
            warm = const_p.tile([1, 16], F32)
            nc.any.memset(warm[:], 0.0)
            warm_o = const_p.tile([1, 16], BF16)
            nc.scalar.activation(warm_o[:], warm[:],
                                 mybir.ActivationFunctionType.Exp, scale=1.0)

            # ---- DMA: >=2KB per-partition lines; round-0 inputs first ----
            # all weights arrive as ONE [128, 2560] DMA (5KB partition lines;
            # per-tile weight DMAs were 256B lines and crawled)
            w_t = w_p.tile([128, 2560], BF16, tag="wpack")
            nc.sync.dma_start(w_t[:], wpack[:, :])
            wq_t = [w_t[:, c * 128:(c + 1) * 128] for c in range(KC_X)]
            wk_t = [w_t[:, 512 + c * 128:512 + (c + 1) * 128] for c in range(KC_C)]
            wv_t = [w_t[:, 1280 + c * 128:1280 + (c + 1) * 128] for c in range(KC_C)]
            wo_t = w_t[:, 2048:2560]

            ctx_t = [ctx_p.tile([128, NKV], BF16, tag=f"ctx{c}", name=f"ctx{c}")
                     for c in range(KC_C)]
            xt_t = [x_p.tile([128, NQ], BF16, tag=f"xt{c}", name=f"xt{c}")
                    for c in range(KC_X)]
            # order: xt whole tiles (4KB lines) for Q, then ctxt first half
            # (K kvb0/kvb1 + early V), then the second half
            for c in range(KC_X):
                nc.sync.dma_start(xt_t[c][:], xt[c * 128:(c + 1) * 128, :])
            for c in range(KC_C):
                nc.sync.dma_start(ctx_t[c][:, 0:1024], ctxt[c * 128:(c + 1) * 128, 0:1024])
            for c in range(KC_C):
                nc.sync.dma_start(ctx_t[c][:, 1024:2048],
                                  ctxt[c * 128:(c + 1) * 128, 1024:2048])

            # ---- projection emitters ----
            qt_t = qt_p.tile([128, NQ], BF16)
            kt_t = kt_p.tile([128, NKV], BF16)
            va_t = va_p.tile([128, NKC * 130], BF16)
            dst_ones = va_t[:].rearrange("p (g c) -> p g c", c=65)[:, :, 64:65]
            nc.vector.tensor_copy(dst_ones, ones32[:, :, None])

            def emit_q_qb(qb):
                ps = proj_ps.tile([128, 512], F32, tag="proj", name=f"pq{qb}")
                for c in range(KC_X):
                    nc.tensor.matmul(ps[:], wq_t[c],
                                     xt_t[c][:, qb * 512:(qb + 1) * 512],
                                     start=(c == 0), stop=(c == KC_X - 1))
                nc.vector.tensor_copy(qt_t[:, qb * 512:(qb + 1) * 512], ps[:])

            def emit_k_kvb(kvb):
                ps = proj_ps.tile([128, 512], F32, tag="proj", name=f"pk{kvb}")
                for c in range(KC_C):
                    nc.tensor.matmul(ps[:], wk_t[c],
                                     ctx_t[c][:, kvb * 512:(kvb + 1) * 512],
                                     start=(c == 0), stop=(c == KC_C - 1))
                nc.vector.tensor_copy(kt_t[:, kvb * 512:(kvb + 1) * 512], ps[:])

            def emit_v_pair(vp):
                # two kv-chunks (2*vp, 2*vp+1) -> one [128, 256] psum region
                ps = proj_ps.tile([128, 512], F32, tag="proj", name=f"pvg{vp}")
                for i in range(2):
                    kvc = vp * 2 + i
                    for c in range(KC_C):
                        nc.tensor.matmul(
                            ps[:, i * 128:(i + 1) * 128],
                            ctx_t[c][:, kvc * 128:(kvc + 1) * 128], wv_t[c],
                            start=(c == 0), stop=(c == KC_C - 1))
                src = ps[:, 0:256].rearrange("p (i h d) -> p i h d", i=2, h=2)
                dst = va_t[:, vp * 260:(vp + 1) * 260]
                dst = dst.rearrange("p (i h d) -> p i h d", i=2, h=2, d=65)[:, :, :, 0:64]
                nc.vector.tensor_copy(dst, src)

            emit_q_qb(0)
            emit_k_kvb(0)
            emit_k_kvb(1)
            for vp in range(2):
                emit_v_pair(vp)

            # fillers after round r of qb0 (kt kvb k ready before S round 2k;
            # va chunk pair g ready before PV round g, which is emitted in
            # round g+1)
            qb0_fill = {
                0: [lambda: emit_v_pair(2), lambda: emit_v_pair(3)],
                1: [lambda: emit_k_kvb(2), lambda: emit_v_pair(4)],
                2: [lambda: emit_v_pair(5)],
                3: [lambda: emit_k_kvb(3), lambda: emit_v_pair(6)],
                4: [lambda: emit_v_pair(7), lambda: emit_q_qb(1)],
                5: [lambda: emit_q_qb(2)],
                6: [lambda: emit_q_qb(3)],
            }

            # ---- attention; norm + out-proj of qb deferred into qb+1 ----
            sps = [sa_ps, sb_ps]

            def make_tail(qb, pv_t):
                recs = [None, None]

                def norm_pre():
                    # DVE-only: denom rows down to partition 0, approx recip
                    dens = []
                    for h in range(2):
                        den = small_p.tile([1, 512], F32, tag="den",
                                           name=f"den{qb}_{h}")
                        nc.vector.tensor_copy(den[:], pv_t[h][64:65, :])
                        dens.append(den)
                    for h in range(2):
                        rec = small_p.tile([1, 512], F32, tag="rec",
                                           name=f"rec{qb}_{h}")
                        nc.vector.reciprocal_approx_fast(rec[:], dens[h][:])
                        rec_b = small_p.tile([1, 512], BF16, tag="recb",
                                             name=f"recb{qb}_{h}")
                        nc.vector.tensor_copy(rec_b[:], rec[:])
                        recs[h] = rec_b

                def norm_fin():
                    at_t = at_p.tile([128, 512], BF16, tag="at", name=f"at{qb}")
                    for h in range(2):
                        ps_b = proj_ps.tile([64, 512], F32, tag="proj",
                                            name=f"psb{qb}_{h}")
                        nc.tensor.matmul(ps_b[:], ones_r[:], recs[h][:],
                                         start=True, stop=True)
                        b_sb = small_p.tile([64, 512], F32, tag="bsb",
                                            name=f"bsb{qb}_{h}")
                        nc.vector.tensor_copy(b_sb[:], ps_b[:])
                        nc.vector.tensor_tensor(at_t[h * 64:(h + 1) * 64, :],
                                                pv_t[h][0:64, :], b_sb[:],
                                                op=mybir.AluOpType.mult)
                    return at_t

                def tail_oproj(at_t, final=False):
                    for m in range(4):
                        ps_o = proj_ps.tile([128, 512], F32, tag="proj",
                                            name=f"po{qb}_{m}")
                        nc.tensor.matmul(ps_o[:], wo_t[:, m * 128:(m + 1) * 128],
                                         at_t[:], start=True, stop=True)
                        o_sb = out_p.tile([128, 512], BF16, tag="osb",
                                          name=f"o{qb}_{m}")
                        if final and m % 2 == 1:
                            # ACT is idle after the last exp; share the copies
                            nc.scalar.copy(o_sb[:], ps_o[:])
                        else:
                            nc.vector.tensor_copy(o_sb[:], ps_o[:])
                        nc.sync.dma_start(
                            otp[m * 128:(m + 1) * 128, qb * 512:(qb + 1) * 512],
                            o_sb[:])

                return [norm_pre, norm_fin, tail_oproj]

            pending_tail = [None]
            tail_at = [None]

            def drain_tail(step):
                # 0: DVE recip prefix; 1: broadcast+mult (before new PV r0);
                # 2: out-projection
                if pending_tail[0] is None:
                    return
                pre, fin, oproj = pending_tail[0]
                if step == 0:
                    pre()
                elif step == 1:
                    tail_at[0] = fin()
                else:
                    oproj(tail_at[0])
                    pending_tail[0] = None

            def s_head(qb, g, h, sp_name):
                sp = sps[h].tile([128, 1024], F32, tag=f"s{h}", name=sp_name)
                for j in range(2):
                    kvc = g * 2 + j
                    nc.tensor.matmul(
                        sp[:, j * 512:(j + 1) * 512],
                        kt_t[h * 64:(h + 1) * 64, kvc * 128:(kvc + 1) * 128],
                        qt_t[h * 64:(h + 1) * 64, qb * 512:(qb + 1) * 512],
                        start=True, stop=True)
                p_t = p_p.tile([128, 1024], BF16, tag="p", name=f"p_{sp_name}")
                nc.scalar.activation(p_t[:], sp[:],
                                     mybir.ActivationFunctionType.Exp, scale=SCALE)
                return p_t

            def pv_head(pv_t, va_col, p_t):
                for j in range(2):
                    kvc = va_col[j]
                    nc.tensor.matmul(
                        pv_t[:],
                        va_t[:, kvc * 130:kvc * 130 + 65],
                        p_t[:, j * 512:(j + 1) * 512],
                        start=(kvc == 0), stop=(kvc == NKC - 1))

            def pv_head_h(pv_t, g, h, p_t):
                for j in range(2):
                    kvc = g * 2 + j
                    nc.tensor.matmul(
                        pv_t[:],
                        va_t[:, kvc * 130 + h * 65:kvc * 130 + (h + 1) * 65],
                        p_t[:, j * 512:(j + 1) * 512],
                        start=(kvc == 0), stop=(kvc == NKC - 1))

            def emit_attn_qb(qb):
                pv_t = [pv_ps.tile([65, 512], F32, tag="pv", name=f"pv{qb}_{h}")
                        for h in range(2)]
                p_prev = None
                for g in range(NRND):
                    p_cur = [s_head(qb, g, 0, f"s{qb}_{g}_0"),
                             s_head(qb, g, 1, f"s{qb}_{g}_1")]
                    if qb == 0 and g in qb0_fill:
                        for f in qb0_fill[g]:
                            f()
                    if g <= 2:
                        drain_tail(g)
                    if p_prev is not None:
                        for h in range(2):
                            pv_head_h(pv_t[h], g - 1, h, p_prev[h])
                    p_prev = p_cur
                for h in range(2):
                    pv_head_h(pv_t[h], NRND - 1, h, p_prev[h])
                pending_tail[0] = make_tail(qb, pv_t)

            for qb in range(NQB):
                emit_attn_qb(qb)
            pre, fin, oproj = pending_tail[0]
            pre()
            oproj(fin(), final=True)

    nc.finalize()
    return nc


def _bf16(a):
    return np.ascontiguousarray(a).astype(NP_BF16)


def run_spmd(inputs, trace=False):
    if "nc" not in _CACHE:
        _CACHE["nc"] = _build_nc()
    nc = _CACHE["nc"]

    x = np.asarray(inputs["x"], dtype=np.float32)
    context = np.asarray(inputs["context"], dtype=np.float32)
    Wq = np.asarray(inputs["Wq"], dtype=np.float32)
    Wk = np.asarray(inputs["Wk"], dtype=np.float32)
    Wv = np.asarray(inputs["Wv"], dtype=np.float32)
    Wo = np.asarray(inputs["Wo"], dtype=np.float32)
    bo = np.asarray(inputs["bo"], dtype=np.float32)

    xt_b = [_bf16(x[b].T) for b in range(B)]
    ctxt_b = [_bf16(context[b].T) for b in range(B)]
    in_maps = []
    for c in range(N_CORES):
        b, p = c // 4, c % 4
        s = slice(p * DP, (p + 1) * DP)
        wpk = np.empty((128, 2560), dtype=NP_BF16)
        for c in range(KC_X):
            wpk[:, c * 128:(c + 1) * 128] = _bf16(Wq[c * 128:(c + 1) * 128, s])
        for c in range(KC_C):
            wpk[:, 512 + c * 128:512 + (c + 1) * 128] = \
                _bf16(Wk[c * 128:(c + 1) * 128, s])
            wpk[:, 1280 + c * 128:1280 + (c + 1) * 128] = \
                _bf16(Wv[c * 128:(c + 1) * 128, s])
        wpk[:, 2048:2560] = _bf16(Wo[s, :])
        in_maps.append({"xt": xt_b[b], "ctxt": ctxt_b[b], "wpack": wpk})

    res = run_bass_kernel_spmd(nc, in_maps, core_ids=list(range(N_CORES)),
                               trace=trace)
    out = np.empty((B, NQ, DI), dtype=np.float32)
    for b in range(B):
        acc = np.zeros((DI, NQ), dtype=np.float32)
        for p in range(4):
            acc += res.results[4 * b + p]["otp"].astype(np.float32)
        out[b] = acc.T + bo[None, :]
    return out, res


def kernel(**inputs):
    out, _ = run_spmd(inputs, trace=False)
    return out


# revision 20
# speedup vs baseline: 1.1875x; 1.0238x over previous
"""Trainium2 Bass kernel for nn_CrossAttention (B=2, Nq=Nk=2048, H=8, Dh=64,
Dx=512, Dctx=768).

Sharding: (batch, head-pair) across 8 cores — core c = (b, p) with b = c//4,
p = c%4 handles heads {2p, 2p+1} of batch b over ALL 2048 queries. K/V/Q
projections cover only the core's 128-wide D_inner slice (4x less projection
work than q-sharding); the output projection emits the PARTIAL product
Wo[128p:128p+128, :]^T @ attn_pair^T which the host sums across the 4 cores
of each batch during unshard (bias bo added on host).

Schedule is paced by ACT-engine exp (~72us/core floor at [128,1024] per
instruction): S = K^T Q runs as row-tiled PE pairs (tile_position
(0,0)/(64,0), both heads concurrent), each head's S lives in its own 2-bank
PSUM tile so next-round S matmuls chase exp bank-pair by bank-pair, softmax
denominators come from a ones-column in the V stationary ([128, 65]), and
normalization uses the custom-DVE approx reciprocal (valid only at partition
base 0 — denom row is first copied down from partition 64). DMA uses >=2KB
per-partition lines (weights packed into one [128, 2560] transfer, whole xt
tiles, ctxt half-tiles); K/V/Q projection tails are spread thinly through
qb0's attention rounds; each qb's normalize + out-projection is deferred in
three stages into the next qb's early rounds so neither ACT nor the PE queue
stalls at block boundaries.
"""

import sys

sys.path.insert(0, "/opt/trn_rl_repo")

import numpy as np
import ml_dtypes

import concourse.bacc as bacc
import concourse.mybir as mybir
import concourse.tile as tile
from concourse.bass_utils import run_bass_kernel_spmd
from contextlib import ExitStack

F32 = mybir.dt.float32
BF16 = mybir.dt.bfloat16
NP_BF16 = np.dtype(ml_dtypes.bfloat16)

B = 2
NQ = 2048
NKV = 2048
DX = 512
DC = 768
DI = 512
NH = 8
DH = 64
DP = 128
N_CORES = 8

KC_X = DX // 128
KC_C = DC // 128
NKC = NKV // 128
NQB = NQ // 512
NRND = NKC // 2
SCALE = DH ** -0.5

_CACHE = {}


def _build_nc():
    nc = bacc.Bacc("TRN2", target_bir_lowering=False, debug=False, num_devices=N_CORES)

    xt = nc.declare_dram_parameter("xt", [DX, NQ], BF16, isOutput=False)
    ctxt = nc.declare_dram_parameter("ctxt", [DC, NKV], BF16, isOutput=False)
    wpack = nc.declare_dram_parameter("wpack", [128, 2560], BF16, isOutput=False)
    otp = nc.declare_dram_parameter("otp", [DI, NQ], BF16, isOutput=True)

    with tile.TileContext(nc) as tc:
        with ExitStack() as ctx:
            # ---- SBUF pools ----
            const_p = ctx.enter_context(tc.tile_pool(name="const", bufs=1))
            w_p = ctx.enter_context(tc.tile_pool(name="weights", bufs=1))
            x_p = ctx.enter_context(tc.tile_pool(name="xt", bufs=1))
            ctx_p = ctx.enter_context(tc.tile_pool(name="ctxt", bufs=1))
            kt_p = ctx.enter_context(tc.tile_pool(name="kt", bufs=1))
            qt_p = ctx.enter_context(tc.tile_pool(name="qt", bufs=1))
            va_p = ctx.enter_context(tc.tile_pool(name="va", bufs=1))
            p_p = ctx.enter_context(tc.tile_pool(name="p", bufs=6))
            at_p = ctx.enter_context(tc.tile_pool(name="at", bufs=2))
            small_p = ctx.enter_context(tc.tile_pool(name="small", bufs=2))
            out_p = ctx.enter_context(tc.tile_pool(name="outsb", bufs=2))
            # ---- PSUM pools: (2+2) + 2 + 2 = 8 banks ----
            sa_ps = ctx.enter_context(tc.tile_pool(name="sa", bufs=1, space="PSUM"))
            sb_ps = ctx.enter_context(tc.tile_pool(name="sb", bufs=1, space="PSUM"))
            pv_ps = ctx.enter_context(tc.tile_pool(name="pv", bufs=2, space="PSUM"))
            proj_ps = ctx.enter_context(tc.tile_pool(name="proj", bufs=2, space="PSUM"))

            # ---- constants; dummy exp preloads the ACT exp table set ----
            ones_f = const_p.tile([1, 64], F32)
            nc.any.memset(ones_f[:], 1.0)
            ones_r = const_p.tile([1, 64], BF16)
            nc.vector.tensor_copy(ones_r[:], ones_f[:])
            ones32 = const_p.tile([128, 32], F32)
            nc.any.memset(ones32[:], 1.0)
            warm = const_p.tile([1, 16], F32)
            nc.any.memset(warm[:], 0.0)
            warm_o = const_p.tile([1, 16], BF16)
            nc.scalar.activation(warm_o[:], warm[:],
                                 mybir.ActivationFunctionType.Exp, scale=1.0)
            # PE heartbeat during the DMA wait: back-to-back matmuls make the
            # HAM activity window see a busy PE so the 2.4GHz unthrottle lands
            # before the projections start (otherwise they run cold)
            hb_f = const_p.tile([1, 512], F32)
            nc.any.memset(hb_f[:], 0.0)
            hb_b = const_p.tile([1, 512], BF16)
            nc.vector.tensor_copy(hb_b[:], hb_f[:])

            # ---- DMA: >=2KB per-partition lines; round-0 inputs first ----
            # all weights arrive as ONE [128, 2560] DMA (5KB partition lines;
            # per-tile weight DMAs were 256B lines and crawled)
            w_t = w_p.tile([128, 2560], BF16, tag="wpack")
            nc.sync.dma_start(w_t[:], wpack[:, :])
            wq_t = [w_t[:, c * 128:(c + 1) * 128] for c in range(KC_X)]
            wk_t = [w_t[:, 512 + c * 128:512 + (c + 1) * 128] for c in range(KC_C)]
            wv_t = [w_t[:, 1280 + c * 128:1280 + (c + 1) * 128] for c in range(KC_C)]
            wo_t = w_t[:, 2048:2560]

            ctx_t = [ctx_p.tile([128, NKV], BF16, tag=f"ctx{c}", name=f"ctx{c}")
                     for c in range(KC_C)]
            xt_t = [x_p.tile([128, NQ], BF16, tag=f"xt{c}", name=f"xt{c}")
                    for c in range(KC_X)]
            # order: xt whole tiles (4KB lines) for Q, then ctxt first half
            # (K kvb0/kvb1 + early V), then the second half
            for c in range(KC_X):
                nc.sync.dma_start(xt_t[c][:], xt[c * 128:(c + 1) * 128, :])
            for c in range(KC_C):
                nc.sync.dma_start(ctx_t[c][:, 0:1024], ctxt[c * 128:(c + 1) * 128, 0:1024])
            for c in range(KC_C):
                nc.sync.dma_start(ctx_t[c][:, 1024:2048],
                                  ctxt[c * 128:(c + 1) * 128, 1024:2048])

            ps_hb = proj_ps.tile([64, 512], F32, tag="proj", name="pshb")
            for _ in range(16):
                nc.tensor.matmul(ps_hb[:], ones_r[:], hb_b[:],
                                 start=True, stop=True)

            # ---- projection emitters ----
            qt_t = qt_p.tile([128, NQ], BF16)
            kt_t = kt_p.tile([128, NKV], BF16)
            va_t = va_p.tile([128, NKC * 130], BF16)
            dst_ones = va_t[:].rearrange("p (g c) -> p g c", c=65)[:, :, 64:65]
            nc.vector.tensor_copy(dst_ones, ones32[:, :, None])

            def emit_q_qb(qb):
                ps = proj_ps.tile([128, 512], F32, tag="proj", name=f"pq{qb}")
                for c in range(KC_X):
                    nc.tensor.matmul(ps[:], wq_t[c],
                                     xt_t[c][:, qb * 512:(qb + 1) * 512],
                                     start=(c == 0), stop=(c == KC_X - 1))
                nc.vector.tensor_copy(qt_t[:, qb * 512:(qb + 1) * 512], ps[:])

            def emit_k_kvb(kvb):
                ps = proj_ps.tile([128, 512], F32, tag="proj", name=f"pk{kvb}")
                for c in range(KC_C):
                    nc.tensor.matmul(ps[:], wk_t[c],
                                     ctx_t[c][:, kvb * 512:(kvb + 1) * 512],
                                     start=(c == 0), stop=(c == KC_C - 1))
                nc.vector.tensor_copy(kt_t[:, kvb * 512:(kvb + 1) * 512], ps[:])

            def emit_v_pair(vp):
                # two kv-chunks (2*vp, 2*vp+1) -> one [128, 256] psum region
                ps = proj_ps.tile([128, 512], F32, tag="proj", name=f"pvg{vp}")
                for i in range(2):
                    kvc = vp * 2 + i
                    for c in range(KC_C):
                        nc.tensor.matmul(
                            ps[:, i * 128:(i + 1) * 128],
                            ctx_t[c][:, kvc * 128:(kvc + 1) * 128], wv_t[c],
                            start=(c == 0), stop=(c == KC_C - 1))
                src = ps[:, 0:256].rearrange("p (i h d) -> p i h d", i=2, h=2)
                dst = va_t[:, vp * 260:(vp + 1) * 260]
                dst = dst.rearrange("p (i h d) -> p i h d", i=2, h=2, d=65)[:, :, :, 0:64]
                nc.vector.tensor_copy(dst, src)

            emit_q_qb(0)
            emit_k_kvb(0)
            emit_k_kvb(1)
            for vp in range(2):
                emit_v_pair(vp)

            # fillers after round r of qb0 (kt kvb k ready before S round 2k;
            # va chunk pair g ready before PV round g, which is emitted in
            # round g+1)
            qb0_fill = {
                0: [lambda: emit_v_pair(2), lambda: emit_v_pair(3)],
                1: [lambda: emit_k_kvb(2), lambda: emit_v_pair(4)],
                2: [lambda: emit_v_pair(5)],
                3: [lambda: emit_k_kvb(3), lambda: emit_v_pair(6)],
                4: [lambda: emit_v_pair(7), lambda: emit_q_qb(1)],
                5: [lambda: emit_q_qb(2)],
                6: [lambda: emit_q_qb(3)],
            }

            # ---- attention; norm + out-proj of qb deferred into qb+1 ----
            sps = [sa_ps, sb_ps]

            def make_tail(qb, pv_t):
                recs = [None, None]

                def norm_pre(final=False):
                    # denom rows down to partition 0, approx recip (DVE);
                    # in the final tail ACT is idle and takes the bf16 casts
                    dens = []
                    for h in range(2):
                        den = small_p.tile([1, 512], F32, tag="den",
                                           name=f"den{qb}_{h}")
                        nc.vector.tensor_copy(den[:], pv_t[h][64:65, :])
                        dens.append(den)
                    for h in range(2):
                        rec = small_p.tile([1, 512], F32, tag="rec",
                                           name=f"rec{qb}_{h}")
                        nc.vector.reciprocal_approx_fast(rec[:], dens[h][:])
                        rec_b = small_p.tile([1, 512], BF16, tag="recb",
                                             name=f"recb{qb}_{h}")
                        if final:
                            nc.scalar.copy(rec_b[:], rec[:])
                        else:
                            nc.vector.tensor_copy(rec_b[:], rec[:])
                        recs[h] = rec_b

                def norm_fin(final=False):
                    at_t = at_p.tile([128, 512], BF16, tag="at", name=f"at{qb}")
                    for h in range(2):
                        ps_b = proj_ps.tile([64, 512], F32, tag="proj",
                                            name=f"psb{qb}_{h}")
                        nc.tensor.matmul(ps_b[:], ones_r[:], recs[h][:],
                                         start=True, stop=True)
                        b_sb = small_p.tile([64, 512], F32, tag="bsb",
                                            name=f"bsb{qb}_{h}")
                        if final:
                            nc.scalar.copy(b_sb[:], ps_b[:])
                        else:
                            nc.vector.tensor_copy(b_sb[:], ps_b[:])
                        nc.vector.tensor_tensor(at_t[h * 64:(h + 1) * 64, :],
                                                pv_t[h][0:64, :], b_sb[:],
                                                op=mybir.AluOpType.mult)
                    return at_t

                def tail_oproj(at_t, final=False):
                    for m in range(4):
                        ps_o = proj_ps.tile([128, 512], F32, tag="proj",
                                            name=f"po{qb}_{m}")
                        nc.tensor.matmul(ps_o[:], wo_t[:, m * 128:(m + 1) * 128],
                                         at_t[:], start=True, stop=True)
                        o_sb = out_p.tile([128, 512], BF16, tag="osb",
                                          name=f"o{qb}_{m}")
                        if final and m % 2 == 1:
                            # ACT is idle after the last exp; share the copies
                            nc.scalar.copy(o_sb[:], ps_o[:])
                        else:
                            nc.vector.tensor_copy(o_sb[:], ps_o[:])
                        nc.sync.dma_start(
                            otp[m * 128:(m + 1) * 128, qb * 512:(qb + 1) * 512],
                            o_sb[:])

                return [norm_pre, norm_fin, tail_oproj]

            pending_tail = [None]
            tail_at = [None]

            def drain_tail(step):
                # 0: DVE recip prefix; 1: broadcast+mult (before new PV r0);
                # 2: out-projection
                if pending_tail[0] is None:
                    return
                pre, fin, oproj = pending_tail[0]
                if step == 0:
                    pre()
                elif step == 1:
                    tail_at[0] = fin()
                else:
                    oproj(tail_at[0])
                    pending_tail[0] = None

            def s_head(qb, g, h, sp_name):
                sp = sps[h].tile([128, 1024], F32, tag=f"s{h}", name=sp_name)
                for j in range(2):
                    kvc = g * 2 + j
                    nc.tensor.matmul(
                        sp[:, j * 512:(j + 1) * 512],
                        kt_t[h * 64:(h + 1) * 64, kvc * 128:(kvc + 1) * 128],
                        qt_t[h * 64:(h + 1) * 64, qb * 512:(qb + 1) * 512],
                        start=True, stop=True)
                p_t = p_p.tile([128, 1024], BF16, tag="p", name=f"p_{sp_name}")
                nc.scalar.activation(p_t[:], sp[:],
                                     mybir.ActivationFunctionType.Exp, scale=SCALE)
                return p_t

            def pv_head(pv_t, va_col, p_t):
                for j in range(2):
                    kvc = va_col[j]
                    nc.tensor.matmul(
                        pv_t[:],
                        va_t[:, kvc * 130:kvc * 130 + 65],
                        p_t[:, j * 512:(j + 1) * 512],
                        start=(kvc == 0), stop=(kvc == NKC - 1))

            def pv_head_h(pv_t, g, h, p_t):
                for j in range(2):
                    kvc = g * 2 + j
                    nc.tensor.matmul(
                        pv_t[:],
                        va_t[:, kvc * 130 + h * 65:kvc * 130 + (h + 1) * 65],
                        p_t[:, j * 512:(j + 1) * 512],
                        start=(kvc == 0), stop=(kvc == NKC - 1))

            pending_pv = [None]

            def emit_attn_qb(qb):
                pv_t = [pv_ps.tile([65, 512], F32, tag="pv", name=f"pv{qb}_{h}")
                        for h in range(2)]
                p_prev = None
                for g in range(NRND):
                    p_cur = [s_head(qb, g, 0, f"s{qb}_{g}_0"),
                             s_head(qb, g, 1, f"s{qb}_{g}_1")]
                    if g == 0 and pending_pv[0] is not None:
                        # previous qb's last PV pair runs here, AFTER this
                        # qb's first S pair, so the last exp of the previous
                        # qb flows straight into this qb's first exp
                        pending_pv[0]()
                        pending_pv[0] = None
                    if qb == 0 and g in qb0_fill:
                        for f in qb0_fill[g]:
                            f()
                    if g <= 2:
                        drain_tail(g)
                    if p_prev is not None:
                        for h in range(2):
                            pv_head_h(pv_t[h], g - 1, h, p_prev[h])
                    p_prev = p_cur

                def last_pv(pv_t=pv_t, p_prev=p_prev):
                    for h in range(2):
                        pv_head_h(pv_t[h], NRND - 1, h, p_prev[h])
                pending_pv[0] = last_pv
                pending_tail[0] = make_tail(qb, pv_t)

            for qb in range(NQB):
                emit_attn_qb(qb)
            pending_pv[0]()
            pre, fin, oproj = pending_tail[0]
            pre(final=True)
            oproj(fin(final=True), final=True)

    nc.finalize()
    return nc


def _bf16(a):
    return np.ascontiguousarray(a).astype(NP_BF16)


def run_spmd(inputs, trace=False):
    if "nc" not in _CACHE:
        _CACHE["nc"] = _build_nc()
    nc = _CACHE["nc"]

    x = np.asarray(inputs["x"], dtype=np.float32)
    context = np.asarray(inputs["context"], dtype=np.float32)
    Wq = np.asarray(inputs["Wq"], dtype=np.float32)
    Wk = np.asarray(inputs["Wk"], dtype=np.float32)
    Wv = np.asarray(inputs["Wv"], dtype=np.float32)
    Wo = np.asarray(inputs["Wo"], dtype=np.float32)
    bo = np.asarray(inputs["bo"], dtype=np.float32)

    xt_b = [_bf16(x[b].T) for b in range(B)]
    ctxt_b = [_bf16(context[b].T) for b in range(B)]
    in_maps = []
    for c in range(N_CORES):
        b, p = c // 4, c % 4
        s = slice(p * DP, (p + 1) * DP)
        wpk = np.empty((128, 2560), dtype=NP_BF16)
        for c in range(KC_X):
            wpk[:, c * 128:(c + 1) * 128] = _bf16(Wq[c * 128:(c + 1) * 128, s])
        for c in range(KC_C):
            wpk[:, 512 + c * 128:512 + (c + 1) * 128] = \
                _bf16(Wk[c * 128:(c + 1) * 128, s])
            wpk[:, 1280 + c * 128:1280 + (c + 1) * 128] = \
                _bf16(Wv[c * 128:(c + 1) * 128, s])
        wpk[:, 2048:2560] = _bf16(Wo[s, :])
        in_maps.append({"xt": xt_b[b], "ctxt": ctxt_b[b], "wpack": wpk})

    res = run_bass_kernel_spmd(nc, in_maps, core_ids=list(range(N_CORES)),
                               trace=trace)
    out = np.empty((B, NQ, DI), dtype=np.float32)
    for b in range(B):
        acc = np.zeros((DI, NQ), dtype=np.float32)
        for p in range(4):
            acc += res.results[4 * b + p]["otp"].astype(np.float32)
        out[b] = acc.T + bo[None, :]
    return out, res


def kernel(**inputs):
    out, _ = run_spmd(inputs, trace=False)
    return out
